# revision 1
# baseline (speedup 1.0000x reference)
"""Trainium2 Bass kernel v2 for nn_CategoricalDecoder (topk_masking).

Phase A (bin-sharded): tail-feature logits + local top-16 (3-term f32r
split matmuls). AllToAll flips to batch sharding. Phase B: merge, gather
winning z rows, exact fp32-class recompute of num/den on the 512 selected
columns, logsumexp.
"""

import numpy as np
from contextlib import ExitStack

import bass_rust as _br
import concourse.bass as bass
import concourse.bacc as bacc
import concourse.tile as tile
from concourse import mybir
from concourse.bass_utils import run_bass_kernel_spmd
from concourse.hw_specs import get_activation_tables

F32 = mybir.dt.float32
F32R = mybir.dt.float32r
U16 = mybir.dt.uint16
I16 = mybir.dt.int16
AF = mybir.ActivationFunctionType
ALU = mybir.AluOpType
AX = mybir.AxisListType

B, N, Lz, H, D, C = 256, 8192, 64, 256, 32, 16
DC = D * C
P = 8
NL = N // P
BL = B // P
K = 16
NEG = -1.0e30

# pk64 column offsets
O_ZTSH, O_ZTSL, O_W1H, O_W1L, O_OHT, O_B2T, O_G4 = (
    0, 1024, 2048, 2304, 2560, 2816, 2817)
PK64_COLS = 2821
# pk128 column offsets
O_W2H, O_W2L, O_B1, O_B2, O_OHB, O_GSEL, O_COEF, O_ONES, O_CO, O_IOTA, O_NCBT = (
    0, 1024, 2048, 2050, 2054, 2182, 2310, 2438, 2440, 2441, 2569)
PK128_COLS = 2570


class _Bacc(bacc.Bacc):
    """Bacc that pins every activation to the one table holding
    {Relu, Exp, Ln, Copy}, avoiding per-switch ACT_TABLE_LOADs."""

    def insert_act_table_loads(self):
        has_act = any(isinstance(i, mybir.InstActivation)
                      for b in self.main_func.blocks for i in b.instructions)
        if not has_act:
            return
        tables = []
        for name, funcs in get_activation_tables(self.m.arch).items():
            keep = funcs if name == "natural_log_exp_and_others" else set()
            tables.append((name, keep))
        _br.insert_act_table_loads(self, tables)


def _build_nc():
    nc = _Bacc("TRN2", target_bir_lowering=False, num_devices=P)

    dp = nc.declare_dram_parameter
    pk64 = dp("pk64", [Lz, PK64_COLS], F32R, isOutput=False)
    pk128 = dp("pk128", [128, PK128_COLS], F32R, isOutput=False)
    ztf = dp("ztf", [Lz, N], F32, isOutput=False)
    outp = dp("out", [BL], F32, isOutput=True)

    with tile.TileContext(nc) as tc, ExitStack() as ctx:
        const = ctx.enter_context(tc.tile_pool(name="const", bufs=1))
        dram = ctx.enter_context(tc.tile_pool(name="dram", bufs=1, space="DRAM"))

        k64 = const.tile([Lz, PK64_COLS], F32R, name="k64")
        nc.sync.dma_start(k64[:], pk64[:])
        k128 = const.tile([128, PK128_COLS], F32R, name="k128")
        nc.sync.dma_start(k128[:], pk128[:])
        ztf_sb = const.tile([Lz, N], F32, name="ztf_sb")
        nc.sync.dma_start(ztf_sb[:], ztf[:])

        def c64(off, w, p=Lz, dt=None):
            ap = k64[0:p, off:off + w]
            return ap.bitcast(dt) if dt else ap

        def c128(off, w, p=128, dt=None):
            ap = k128[0:p, off:off + w]
            return ap.bitcast(dt) if dt else ap

        xin = dram.tile([B, 16], F32)
        xout = dram.tile([B, 16], F32)

        # early dummy ap_gather: forces the gpsimd gather library load to
        # overlap the parameter DMAs instead of stalling phase B.
        with ExitStack() as ctx0:
            pre = ctx0.enter_context(tc.tile_pool(name="pre", bufs=1))
            zidx = pre.tile([16, 1], I16, name="zidx")
            nc.vector.memset(zidx[:], 0)
            junkg = pre.tile([16, 16], F32, name="junkg")
            nc.gpsimd.ap_gather(junkg[:], k64[0:16, 0:64].bitcast(F32), zidx[:],
                                channels=16, num_elems=64, d=1, num_idxs=16)

        # ================= phase A =================
        with ExitStack() as ctxA:
            pa = ctxA.enter_context(tc.tile_pool(name="pa", bufs=3, space="PSUM"))
            sp = ctxA.enter_context(tc.tile_pool(name="sp", bufs=1, space="PSUM"))
            act = ctxA.enter_context(tc.tile_pool(name="actA", bufs=1))
            scratch = ctxA.enter_context(tc.tile_pool(name="scrA", bufs=1))

            # hT = relu(W1.T @ zT + b1), 3-term f32r
            hs = []
            for m in range(2):
                ph = pa.tile([128, NL], F32, tag="mm")
                for f in range(2):
                    sl = slice(f * 512, (f + 1) * 512)
                    w1h = c64(O_W1H + m * 128, 128)
                    w1l = c64(O_W1L + m * 128, 128)
                    zh = c64(O_ZTSH + f * 512, 512)
                    zl = c64(O_ZTSL + f * 512, 512)
                    nc.tensor.matmul(ph[:, sl], w1h, zh, start=True, stop=False)
                    nc.tensor.matmul(ph[:, sl], w1h, zl, start=False, stop=False)
                    nc.tensor.matmul(ph[:, sl], w1l, zh, start=False, stop=True)
                b1 = c128(O_B1 + m, 1, dt=F32)
                hh = act.tile([128, NL], F32R, name=f"hh{m}")
                nc.scalar.activation(hh[:], ph[:], AF.Relu, bias=b1)
                hf = act.tile([128, NL], F32, name=f"hf{m}")
                nc.scalar.activation(hf[:], ph[:], AF.Relu, bias=b1)
                hl = act.tile([128, NL], F32R, name=f"hl{m}")
                nc.vector.tensor_sub(hl[:], hf[:], hh[:].bitcast(F32))
                hs.append((hh, hl))

            # tail logits (dc 448..512): [64, NL] (b2 folded out on host)
            pl3 = pa.tile([128, NL], F32, tag="mm")
            for f in range(2):
                sl = slice(f * 512, (f + 1) * 512)
                for kk in range(2):
                    w2h = c128(O_W2H + kk * DC + 448, 64)
                    w2l = c128(O_W2L + kk * DC + 448, 64)
                    hh, hl = hs[kk]
                    nc.tensor.matmul(pl3[0:64, sl], w2h, hh[:, sl],
                                     start=(kk == 0), stop=False)
                    nc.tensor.matmul(pl3[0:64, sl], w2h, hl[:, sl],
                                     start=False, stop=False)
                    nc.tensor.matmul(pl3[0:64, sl], w2l, hh[:, sl],
                                     start=False, stop=(kk == 1))
            b2t = c64(O_B2T, 1, dt=F32)
            e3r = act.tile([Lz, NL], F32R, name="e3r")
            nc.scalar.activation(e3r[:], pl3[0:64, :], AF.Exp, bias=b2t)
            l3h = act.tile([Lz, NL], F32R, name="l3h")
            nc.scalar.copy(l3h[:], pl3[0:64, :])
            l3l = act.tile([Lz, NL], F32R, name="l3l")
            nc.vector.tensor_sub(l3l[:], pl3[0:64, :], l3h[:].bitcast(F32))

            # log-sumexp of the 4 tail feature groups
            pse4 = sp.tile([4, NL], F32, tag="se")
            for f in range(2):
                sl = slice(f * 512, (f + 1) * 512)
                nc.tensor.matmul(pse4[:, sl], c64(O_G4, 4), e3r[:, sl],
                                 start=True, stop=True)
            l4h = act.tile([4, NL], F32R, name="l4h")
            nc.scalar.activation(l4h[:], pse4[:], AF.Ln)
            l4f = act.tile([4, NL], F32, name="l4f")
            nc.scalar.activation(l4f[:], pse4[:], AF.Ln)
            l4l = act.tile([4, NL], F32R, name="l4l")
            nc.vector.tensor_sub(l4l[:], l4f[:], l4h[:].bitcast(F32))

            # tail scores st[bt] [128, NL] = oht.T @ logits3 - sum(l4)
            for bt in range(2):
                pst = pa.tile([128, NL], F32, tag="mm")
                for f in range(2):
                    sl = slice(f * 512, (f + 1) * 512)
                    oht = c64(O_OHT + bt * 128, 128)
                    nc.tensor.matmul(pst[:, sl], oht, l3h[:, sl],
                                     start=True, stop=False)
                    nc.tensor.matmul(pst[:, sl], oht, l3l[:, sl],
                                     start=False, stop=False)
                    nc.tensor.matmul(pst[:, sl], c128(O_COEF, 128, p=4),
                                     l4h[:, sl], start=False, stop=False)
                    nc.tensor.matmul(pst[:, sl], c128(O_COEF, 128, p=4),
                                     l4l[:, sl], start=False, stop=True)

                # local top-8 + global ids, straight from PSUM
                x_sb = act.tile([128, 16], F32, name=f"x{bt}")
                nc.vector.max(x_sb[:, 0:8], pst[:])
                pu = act.tile([128, 8], U16, name=f"pu{bt}")
                nc.vector.max_index(pu[:], x_sb[:, 0:8], pst[:])
                nc.vector.tensor_copy(x_sb[:, 8:16], pu[:])
                nc.vector.tensor_scalar_add(x_sb[:, 8:16], x_sb[:, 8:16],
                                            c128(O_CO, 1, dt=F32))
                nc.sync.dma_start(xin[bt * 128:(bt + 1) * 128, :], x_sb[:])

        nc.gpsimd.collective_compute(
            "AllToAll", ALU.bypass, replica_groups=[list(range(P))],
            ins=[xin[:].opt()], outs=[xout[:].opt()],
        )

        # ================= phase B =================
        with ExitStack() as ctxB:
            pb = ctxB.enter_context(tc.tile_pool(name="pb", bufs=4, space="PSUM"))
            spb = ctxB.enter_context(tc.tile_pool(name="spb", bufs=1, space="PSUM"))
            act = ctxB.enter_context(tc.tile_pool(name="actB", bufs=1))
            scratch = ctxB.enter_context(tc.tile_pool(name="scrB", bufs=1))

            y = act.tile([BL, P, 16], F32, name="y")
            nc.sync.dma_start(y[:], xout[:].rearrange("(s p) f -> p s f", s=P))
            cands = act.tile([BL, P * 8], F32, name="cands")
            nc.vector.tensor_copy(
                cands[:].rearrange("p (a b) -> p a b", a=P), y[:, :, 0:8])
            idxc = act.tile([BL, P * 8], F32, name="idxc")
            nc.vector.tensor_copy(
                idxc[:].rearrange("p (a b) -> p a b", a=P), y[:, :, 8:16])

            wv = act.tile([BL, 16], F32, name="wv")
            nc.vector.max(wv[:, 0:8], cands[:])
            cm = act.tile([BL, P * 8], F32, name="cm")
            nc.vector.match_replace(cm[:], wv[:, 0:8], cands[:], NEG)
            nc.vector.max(wv[:, 8:16], cm[:])
            pw = act.tile([BL, 16], U16, name="pw")
            nc.vector.max_index(pw[:, 0:8], wv[:, 0:8], cands[:])
            nc.vector.max_index(pw[:, 8:16], wv[:, 8:16], cm[:])
            posf = act.tile([BL, 16], F32, name="posf")
            nc.vector.tensor_copy(posf[:], pw[:])

            widp = act.tile([32, 32], F32, name="widp")
            for j in range(16):
                junk = scratch.tile([BL, P * 8], F32, tag="junk")
                nc.vector.scalar_tensor_tensor(
                    junk[:], c128(O_IOTA, P * 8, p=BL, dt=F32), posf[:, j:j + 1],
                    idxc[:], op0=ALU.is_equal, op1=ALU.mult,
                    accum_out=widp[0:BL, j:j + 1])
            tp = act.tile([32, 32], F32, name="tp")
            nc.vector.transpose(tp[:], widp[:])
            idx64 = act.tile([Lz, 32], I16, name="idx64")
            nc.vector.tensor_copy(idx64[0:16, :], tp[0:16, :])
            for g in range(1, 4):
                nc.sync.dma_start(idx64[16 * g:16 * (g + 1), :], idx64[0:16, :])

            ztop = act.tile([Lz, 512], F32, name="ztop")
            nc.gpsimd.ap_gather(ztop[:], ztf_sb[:], idx64[:],
                                channels=Lz, num_elems=N, d=1, num_idxs=512)
            zh = act.tile([Lz, 512], F32R, name="zh")
            nc.vector.tensor_copy(zh[:], ztop[:])
            zl = act.tile([Lz, 512], F32R, name="zl")
            nc.vector.tensor_sub(zl[:], ztop[:], zh[:].bitcast(F32))

            h2s = []
            for m in range(2):
                ph2 = pb.tile([128, 512], F32, tag="mmb")
                w1h = c64(O_W1H + m * 128, 128)
                w1l = c64(O_W1L + m * 128, 128)
                nc.tensor.matmul(ph2[:], w1h, zh[:], start=True, stop=False)
                nc.tensor.matmul(ph2[:], w1h, zl[:], start=False, stop=False)
                nc.tensor.matmul(ph2[:], w1l, zh[:], start=False, stop=True)
                b1 = c128(O_B1 + m, 1, dt=F32)
                hh = act.tile([128, 512], F32R, name=f"hh2{m}")
                nc.scalar.activation(hh[:], ph2[:], AF.Relu, bias=b1)
                hf = act.tile([128, 512], F32, name=f"hf2{m}")
                nc.scalar.activation(hf[:], ph2[:], AF.Relu, bias=b1)
                hl = act.tile([128, 512], F32R, name=f"hl2{m}")
                nc.vector.tensor_sub(hl[:], hf[:], hh[:].bitcast(F32))
                h2s.append((hh, hl))

            pse2 = spb.tile([32, 512], F32, tag="seb")
            lin2s = []
            for t in range(4):
                pl2 = pb.tile([128, 512], F32, tag="mmb")
                for kk in range(2):
                    w2h = c128(O_W2H + kk * DC + t * 128, 128)
                    w2l = c128(O_W2L + kk * DC + t * 128, 128)
                    hh, hl = h2s[kk]
                    nc.tensor.matmul(pl2[:], w2h, hh[:], start=(kk == 0), stop=False)
                    nc.tensor.matmul(pl2[:], w2h, hl[:], start=False, stop=False)
                    nc.tensor.matmul(pl2[:], w2l, hh[:], start=False, stop=(kk == 1))
                b2 = c128(O_B2 + t, 1, dt=F32)
                e2r = act.tile([128, 512], F32R, name=f"e2r{t}")
                nc.scalar.activation(e2r[:], pl2[:], AF.Exp, bias=b2)
                lh = act.tile([128, 512], F32R, name=f"l2h{t}")
                nc.scalar.copy(lh[:], pl2[:])
                ll = act.tile([128, 512], F32R, name=f"l2l{t}")
                nc.vector.tensor_sub(ll[:], pl2[:], lh[:].bitcast(F32))
                lin2s.append((lh, ll))
                nc.tensor.matmul(pse2[:], c128(O_GSEL + t * 32, 32), e2r[:],
                                 start=(t == 0), stop=(t == 3))
            lgh = act.tile([32, 512], F32R, name="lgh")
            nc.scalar.activation(lgh[:], pse2[:], AF.Ln)
            lgf = act.tile([32, 512], F32, name="lgf")
            nc.scalar.activation(lgf[:], pse2[:], AF.Ln)
            lgl = act.tile([32, 512], F32R, name="lgl")
            nc.vector.tensor_sub(lgl[:], lgf[:], lgh[:].bitcast(F32))

            pnum = pb.tile([BL, 512], F32, tag="mmb")
            for t in range(4):
                lh, ll = lin2s[t]
                ohb = c128(O_OHB + t * BL, BL)
                nc.tensor.matmul(pnum[:], ohb, lh[:], start=(t == 0), stop=False)
                nc.tensor.matmul(pnum[:], ohb, ll[:], start=False, stop=False)
            nc.tensor.matmul(pnum[:], c128(O_COEF, BL, p=32), lgh[:],
                             start=False, stop=False)
            nc.tensor.matmul(pnum[:], c128(O_COEF, BL, p=32), lgl[:],
                             start=False, stop=True)
            numfull = act.tile([BL, 512], F32, name="numfull")
            nc.vector.tensor_copy(numfull[:], pnum[:])
            dscr = dram.tile([BL, 512], F32)
            nc.sync.dma_start(dscr[:], numfull[:])
            numd = act.tile([BL, 16], F32, name="numd")
            diag = bass.AP(tensor=dscr[:].tensor, offset=0,
                           ap=[[512 + 16, BL], [1, 16]])
            nc.sync.dma_start(numd[:], diag)

            # den = (numd + (-cbt)) - wv   (cbt: host-side tail-bias fold)
            den = act.tile([BL, 16], F32, name="den")
            nc.vector.scalar_tensor_tensor(
                den[:], numd[:], c128(O_NCBT, 1, p=BL, dt=F32), wv[:],
                op0=ALU.add, op1=ALU.subtract)
            ng = act.tile([BL, 2], F32, name="ng")
            nc.vector.tensor_reduce(ng[:, 0:1], numd[:], axis=AX.X, op=ALU.max,
                                    negate=True)
            nc.vector.tensor_reduce(ng[:, 1:2], den[:], axis=AX.X, op=ALU.max,
                                    negate=True)
            s2 = act.tile([BL, 2], F32, name="s2")
            en = scratch.tile([BL, 16], F32, tag="ex")
            nc.scalar.activation(en[:], numd[:], AF.Exp, bias=ng[:, 0:1],
                                 accum_out=s2[:, 0:1])
            ed = scratch.tile([BL, 16], F32, tag="ex")
            nc.scalar.activation(ed[:], den[:], AF.Exp, bias=ng[:, 1:2],
                                 accum_out=s2[:, 1:2])
            lg = act.tile([BL, 2], F32, name="lg")
            nc.scalar.activation(lg[:], s2[:], AF.Ln)
            t1 = act.tile([BL, 1], F32, name="t1")
            nc.vector.tensor_sub(t1[:], lg[:, 0:1], lg[:, 1:2])
            t2 = act.tile([BL, 1], F32, name="t2")
            nc.vector.tensor_sub(t2[:], ng[:, 1:2], ng[:, 0:1])
            t3 = act.tile([BL, 1], F32, name="t3")
            nc.vector.tensor_add(t3[:], t1[:], t2[:])
            nc.sync.dma_start(outp[:], t3[:, 0])

    nc.compile()
    return nc


def _trunc_split(a):
    a = np.ascontiguousarray(a, np.float32)
    hi = (a.view(np.uint32) & np.uint32(0xFFFFF000)).view(np.float32)
    lo = a - hi
    return hi, lo


def _host_prep(x, z, W1, b1, W2, b2):
    oh = np.zeros((B, DC), np.float32)
    oh[np.arange(B)[:, None], np.arange(D)[None, :] * C + x] = 1.0
    ohT = np.ascontiguousarray(oh.T)
    w2s = np.ascontiguousarray(
        W2.reshape(2, 128, DC).transpose(1, 0, 2).reshape(128, 2 * DC))
    w2h, w2l = _trunc_split(w2s)
    w1h, w1l = _trunc_split(W1)
    cbt = oh[:, 448:512] @ b2[448:512]          # (256,)

    k64c = np.zeros((Lz, PK64_COLS), np.float32)
    k64c[:, O_W1H:O_W1H + H] = w1h
    k64c[:, O_W1L:O_W1L + H] = w1l
    k64c[:, O_OHT:O_OHT + B] = ohT[448:512, :]
    k64c[:, O_B2T] = b2[448:512]
    g4 = np.zeros((Lz, 4), np.float32)
    g4[np.arange(Lz), np.arange(Lz) // 16] = 1.0
    k64c[:, O_G4:O_G4 + 4] = g4

    k128c = np.zeros((128, PK128_COLS), np.float32)
    k128c[:, O_W2H:O_W2H + 2 * DC] = w2h
    k128c[:, O_W2L:O_W2L + 2 * DC] = w2l
    k128c[:, O_B1:O_B1 + 2] = b1.reshape(2, 128).T
    k128c[:, O_B2:O_B2 + 4] = b2.reshape(4, 128).T
    p_idx = np.arange(128)
    for t in range(4):
        k128c[p_idx, O_GSEL + t * 32 + t * 8 + p_idx // 16] = 1.0
    k128c[0:32, O_COEF:O_COEF + 128] = -1.0
    k128c[:, O_ONES] = 1.0
    k128c[0:BL, O_IOTA:O_IOTA + 128] = np.arange(128, dtype=np.float32)[None, :]

    ztfull = np.ascontiguousarray(z.T)
    in_maps = []
    for c in range(P):
        kc64 = k64c.copy()
        zsh, zsl = _trunc_split(z[c * NL:(c + 1) * NL, :].T)
        kc64[:, O_ZTSH:O_ZTSH + NL] = zsh
        kc64[:, O_ZTSL:O_ZTSL + NL] = zsl
        kc128 = k128c.copy()
        kc128[:, O_CO] = c * NL
        for t in range(4):
            kc128[:, O_OHB + t * BL:O_OHB + (t + 1) * BL] = \
                ohT[t * 128:(t + 1) * 128, c * BL:(c + 1) * BL]
        kc128[0:BL, O_NCBT] = -cbt[c * BL:(c + 1) * BL]
        in_maps.append(dict(pk64=kc64, pk128=kc128, ztf=ztfull))
    return in_maps


_NC_CACHE = {}


def kernel(x, log_w, z, k, W1, b1, W2, b2, _trace=False, _trace_kwargs=None):
    assert int(k) == K
    in_maps = _host_prep(np.asarray(x, np.int32), np.asarray(z, np.float32),
                         np.asarray(W1, np.float32), np.asarray(b1, np.float32),
                         np.asarray(W2, np.float32), np.asarray(b2, np.float32))
    if "nc" not in _NC_CACHE:
        _NC_CACHE["nc"] = _build_nc()
    nc = _NC_CACHE["nc"]
    res = run_bass_kernel_spmd(
        nc, in_maps, list(range(P)), trace=_trace, **(_trace_kwargs or {}))
    if _trace:
        _NC_CACHE["last_result"] = res
    return np.concatenate([np.asarray(res.results[c]["out"], np.float32)
                           for c in range(P)])



# revision 9
# speedup vs baseline: 1.1305x; 1.1305x over previous
"""Trainium2 Bass kernel v3 for nn_CategoricalDecoder (topk_masking).

Bin-sharded single-pass design: each core computes full logits for its
1024 bins (f32r 1-term), derives num = full-feature logp sum and
score = tail-feature logp sum for all 256 batch rows via one-hot
matmuls, packs (20-bit fixed-point score key | 12-bit quantized num)
into positive fp32 bit patterns, and max8 extracts the per-row local
top-8 candidates WITH their num payloads in one instruction. An 8KB
AllToAll flips to batch sharding; the receiving core thresholds at the
16th-largest key and computes both logsumexps from the decoded
payloads. No z gather, no second net pass.
"""

import numpy as np
from contextlib import ExitStack

import bass_rust as _br
import concourse.bass as bass
import concourse.bacc as bacc
import concourse.tile as tile
from concourse import mybir
from concourse.bass_utils import run_bass_kernel_spmd
from concourse.hw_specs import get_activation_tables

F32 = mybir.dt.float32
F32R = mybir.dt.float32r
I32 = mybir.dt.int32
U8 = mybir.dt.uint8
AF = mybir.ActivationFunctionType
ALU = mybir.AluOpType
AX = mybir.AxisListType

B, N, Lz, H, D, C = 256, 8192, 64, 256, 32, 16
DC = D * C
P = 8
NL = N // P
BL = B // P
K = 16
NEG = -1.0

# packing constants
KEY_OFF, KEY_SCALE = 24.0, 16384.0
NUM_LO, NUM_W = -140.0, 80.0
QS = 4095.0 / NUM_W
M1, M2 = -86.0, -72.0  # fixed logsumexp shifts (num / den)

# kz column offsets (64-partition tile)
O_ZT, O_W1 = 0, NL
KZ_COLS = NL + H

# k128 column offsets (128-partition tile)
O_W2S = 0                    # 8 x [128,128] f32r: (t,kk) -> (t*2+kk)*128
O_OHS = O_W2S + 1024         # 8 x [128,128] f32r: (t,bt) -> (t*2+bt)*128
O_OHT = O_OHS + 1024         # 2 x [128,128] f32r: tail one-hot (rows 64:128)
O_GSEL = O_OHT + 256         # 4 x [128,32] f32r
O_COEF = O_GSEL + 128        # [32,128] of -1
O_COEFT = O_COEF + 128       # [32,128], -1 on rows 28:32 only
O_B1 = O_COEFT + 128         # [128,2] b1 per m
O_B2 = O_B1 + 2              # [128,4] b2 per t
O_QOFF = O_B2 + 4            # [128,2] q-affine bias per bt
O_KOFF = O_QOFF + 2          # [128,2] key-affine bias per bt
O_C4096 = O_KOFF + 2         # [128,1] int32 4096 (bit pattern)
O_CFFF = O_C4096 + 1         # [128,1] int32 0xFFF
O_ENB = O_CFFF + 1           # [128,1] f32: NUM_LO - M1 (e_n exp bias)
K128_COLS = O_ENB + 1


class _Bacc(bacc.Bacc):
    """Bacc pinning activations to the one table holding
    {Relu, Exp, Ln, Copy}, avoiding per-switch ACT_TABLE_LOADs."""

    def insert_act_table_loads(self):
        has_act = any(isinstance(i, mybir.InstActivation)
                      for b in self.main_func.blocks for i in b.instructions)
        if not has_act:
            return
        tables = []
        for name, funcs in get_activation_tables(self.m.arch).items():
            keep = funcs if name == "natural_log_exp_and_others" else set()
            tables.append((name, keep))
        _br.insert_act_table_loads(self, tables)


def _build_nc():
    nc = _Bacc("TRN2", target_bir_lowering=False, num_devices=P)

    dp = nc.declare_dram_parameter
    kz = dp("kz", [Lz, KZ_COLS], F32R, isOutput=False)
    k128 = dp("k128", [128, K128_COLS], F32R, isOutput=False)
    outp = dp("out", [BL], F32, isOutput=True)
    dxv = dp("dxv", [B, 8], F32, isOutput=True)
    dy = dp("dy", [BL, P * 8], F32, isOutput=True)
    ds2 = dp("ds2", [BL, 2], F32, isOutput=True)
    dw = dp("dw", [BL, 16], F32, isOutput=True)
    dqd = dp("dqd", [BL, P * 8], I32, isOutput=True)
    dkm = dp("dkm", [BL, P * 8], I32, isOutput=True)
    dqf = dp("dqf", [BL, P * 8], F32, isOutput=True)
    ddd = dp("ddd", [BL, P * 8], F32, isOutput=True)
    den_o = dp("den_o", [BL, P * 8], F32, isOutput=True)
    ded_o = dp("ded_o", [BL, P * 8], F32, isOutput=True)

    with tile.TileContext(nc) as tc, ExitStack() as ctx:
        const = ctx.enter_context(tc.tile_pool(name="const", bufs=1))
        dram = ctx.enter_context(tc.tile_pool(name="dram", bufs=1, space="DRAM"))

        kz_sb = const.tile([Lz, KZ_COLS], F32R, name="kz_sb")
        nc.sync.dma_start(kz_sb[:], kz[:])
        k128_sb = const.tile([128, K128_COLS], F32R, name="k128_sb")
        nc.sync.dma_start(k128_sb[:], k128[:])

        def c128(off, w, p=128, dt=None):
            ap = k128_sb[0:p, off:off + w]
            return ap.bitcast(dt) if dt else ap

        xin = dram.tile([B, 8], F32)
        xout = dram.tile([B, 8], F32)
        bar_in = dram.tile([1, 128], U8)
        bar_out = dram.tile([P, 128], U8)

        # early dummy collective: absorbs CC bootstrap + launch skew while
        # the parameter DMAs and phase-A matmuls run.
        nc.gpsimd.collective_compute(
            "AllGather", ALU.bypass, replica_groups=[list(range(P))],
            ins=[bar_in[:].opt()], outs=[bar_out[:].opt()],
        )

        act = ctx.enter_context(tc.tile_pool(name="act", bufs=1))
        scr = ctx.enter_context(tc.tile_pool(name="scr", bufs=6))

        # ---- h = relu(W1.T @ zT + b1): [256, NL] as 2 m-tiles ----
        hrs = []
        with ExitStack() as ctxh:
            hp = ctxh.enter_context(tc.tile_pool(name="hp", bufs=2, space="PSUM"))
            for m in range(2):
                ph = hp.tile([128, NL], F32, tag="h")
                for f in range(2):
                    sl = slice(f * 512, (f + 1) * 512)
                    nc.tensor.matmul(ph[:, sl],
                                     kz_sb[:, O_W1 + m * 128:O_W1 + (m + 1) * 128],
                                     kz_sb[:, sl], start=True, stop=True)
                hr = act.tile([128, NL], F32R, name=f"hr{m}")
                nc.scalar.activation(hr[:], ph[:], AF.Relu,
                                     bias=c128(O_B1 + m, 1, dt=F32))
                hrs.append(hr)

        # ---- per f-chunk: logits, lse, num, score, pack ----
        packed = [act.tile([128, NL], I32, name=f"pk{bt}") for bt in range(2)]
        c4096_t = act.tile([128, 512], I32, name="c4096_t")
        nc.vector.memset(c4096_t[:], 4096)
        with ExitStack() as ctxA:
            lgp = ctxA.enter_context(tc.tile_pool(name="lgp", bufs=2, space="PSUM"))
            nump = ctxA.enter_context(tc.tile_pool(name="nump", bufs=4, space="PSUM"))
            psep = ctxA.enter_context(tc.tile_pool(name="psep", bufs=2, space="PSUM"))

            for f in range(2):
                sl = slice(f * 512, (f + 1) * 512)
                pnum = [nump.tile([128, 512], F32, tag="nm", name=f"pn{f}{i}")
                        for i in range(2)]
                pscore = [nump.tile([128, 512], F32, tag="nm", name=f"ps{f}{i}")
                          for i in range(2)]
                pse = psep.tile([32, 512], F32, tag="se")
                for ti, t in enumerate([3, 0, 1, 2]):
                    lg = lgp.tile([128, 512], F32, tag="lg")
                    for kk in range(2):
                        nc.tensor.matmul(
                            lg[:], c128(O_W2S + (t * 2 + kk) * 128, 128),
                            hrs[kk][:, sl], start=(kk == 0), stop=(kk == 1))
                    e_t = scr.tile([128, 512], F32R, tag="e")
                    nc.scalar.activation(e_t[:], lg[:], AF.Exp,
                                         bias=c128(O_B2 + t, 1, dt=F32))
                    l_t = scr.tile([128, 512], F32R, tag="l")
                    nc.scalar.copy(l_t[:], lg[:])
                    nc.tensor.matmul(pse[:], c128(O_GSEL + t * 32, 32), e_t[:],
                                     start=(ti == 0), stop=(ti == 3))
                    for bt in range(2):
                        nc.tensor.matmul(pnum[bt][:],
                                         c128(O_OHS + (t * 2 + bt) * 128, 128),
                                         l_t[:], start=(ti == 0), stop=False)
                    if t == 3:
                        for bt in range(2):
                            nc.tensor.matmul(pscore[bt][:],
                                             c128(O_OHT + bt * 128, 128),
                                             l_t[:], start=True, stop=False)
                lnp = scr.tile([32, 512], F32R, tag="ln")
                nc.scalar.activation(lnp[:], pse[:], AF.Ln)
                for bt in range(2):
                    nc.tensor.matmul(pnum[bt][:], c128(O_COEF, 128, p=32),
                                     lnp[:], start=False, stop=True)
                    nc.tensor.matmul(pscore[bt][:], c128(O_COEFT, 128, p=32),
                                     lnp[:], start=False, stop=True)
                # pack: q = int(pnum*QS + qoff), key = int(pscore*KS + koff),
                # packed = key*4096 + q  (int32 domain)
                for bt in range(2):
                    q_i = scr.tile([128, 512], I32, tag="qi")
                    nc.vector.tensor_scalar(q_i[:], pnum[bt][:], QS,
                                            c128(O_QOFF + bt, 1, dt=F32),
                                            op0=ALU.mult, op1=ALU.add)
                    k_i = scr.tile([128, 512], I32, tag="ki")
                    nc.vector.tensor_scalar(k_i[:], pscore[bt][:], KEY_SCALE,
                                            c128(O_KOFF + bt, 1, dt=F32),
                                            op0=ALU.mult, op1=ALU.add)
                    k4 = scr.tile([128, 512], I32, tag="k4")
                    nc.vector.tensor_tensor(k4[:], k_i[:], c4096_t[:],
                                            op=ALU.mult)
                    nc.vector.tensor_tensor(packed[bt][:, sl], k4[:], q_i[:],
                                            op=ALU.bitwise_or)

        # ---- local top-8 by packed value; ship via AllToAll ----
        for bt in range(2):
            xv = act.tile([128, 8], F32, name=f"xv{bt}")
            nc.vector.max(xv[:], packed[bt][:].bitcast(F32))
            nc.sync.dma_start(xin[bt * 128:(bt + 1) * 128, :], xv[:])
            nc.sync.dma_start(dxv[bt * 128:(bt + 1) * 128, :], xv[:])

        nc.gpsimd.collective_compute(
            "AllToAll", ALU.bypass, replica_groups=[list(range(P))],
            ins=[xin[:].opt()], outs=[xout[:].opt()],
        )

        # ---- merge: threshold at 16th largest, masked logsumexps ----
        y3 = act.tile([BL, P, 8], F32, name="y3")
        nc.sync.dma_start(y3[:], xout[:].rearrange("(s r) c -> r s c", s=P))
        y = y3[:].rearrange("r s c -> r (s c)")
        nc.sync.dma_start(dy[:], y)
        w1 = act.tile([BL, 8], F32, name="w1")
        nc.vector.max(w1[:], y)
        y2 = act.tile([BL, P * 8], F32, name="y2")
        nc.vector.match_replace(y2[:], w1[:], y, NEG)
        w2 = act.tile([BL, 8], F32, name="w2")
        nc.vector.max(w2[:], y2[:])
        nc.sync.dma_start(dw[:, 0:8], w1[:])
        nc.sync.dma_start(dw[:, 8:16], w2[:])

        u_i = y.bitcast(I32)
        cfff_t = act.tile([BL, P * 8], I32, name="cfff_t")
        nc.vector.memset(cfff_t[:], 0xFFF)
        q_d = act.tile([BL, P * 8], I32, name="qd")
        nc.vector.tensor_tensor(q_d[:], u_i, cfff_t[:], op=ALU.bitwise_and)
        km_i = act.tile([BL, P * 8], I32, name="km")
        nc.vector.tensor_tensor(km_i[:], u_i, q_d[:], op=ALU.subtract)
        q_f = act.tile([BL, P * 8], F32, name="qf")
        nc.vector.tensor_copy(q_f[:], q_d[:])
        km_f = act.tile([BL, P * 8], F32, name="kmf")
        nc.vector.tensor_copy(km_f[:], km_i[:])
        nc.sync.dma_start(dqd[:], q_d[:])
        nc.sync.dma_start(dkm[:], km_i[:])
        nc.sync.dma_start(dqf[:], q_f[:])

        # e_n = exp(num - M1), num = q/QS + NUM_LO
        e_n = act.tile([BL, P * 8], F32, name="en")
        nc.scalar.activation(e_n[:], q_f[:], AF.Exp,
                             scale=1.0 / QS, bias=c128(O_ENB, 1, p=BL, dt=F32))
        # den - M2 = q/QS - km/(4096*KS) + (NUM_LO + KEY_OFF - M2)
        nd = act.tile([BL, P * 8], F32, name="nd")
        nc.vector.tensor_scalar(nd[:], q_f[:], 1.0 / QS,
                                float(NUM_LO + KEY_OFF - M2),
                                op0=ALU.mult, op1=ALU.add)
        dd = act.tile([BL, P * 8], F32, name="dd")
        nc.vector.scalar_tensor_tensor(
            dd[:], km_f[:], -1.0 / (4096.0 * KEY_SCALE), nd[:],
            op0=ALU.mult, op1=ALU.add)
        nc.sync.dma_start(ddd[:], dd[:])
        e_d = act.tile([BL, P * 8], F32, name="ed")
        nc.scalar.activation(e_d[:], dd[:], AF.Exp)
        nc.sync.dma_start(den_o[:], e_n[:])
        nc.sync.dma_start(ded_o[:], e_d[:])

        s2 = act.tile([BL, 2], F32, name="s2")
        junk = act.tile([BL, P * 8], F32, name="junk")
        nc.vector.scalar_tensor_tensor(
            junk[:], y, w2[:, 7:8], e_n[:],
            op0=ALU.is_ge, op1=ALU.mult, accum_out=s2[:, 0:1])
        junk2 = act.tile([BL, P * 8], F32, name="junk2")
        nc.vector.scalar_tensor_tensor(
            junk2[:], y, w2[:, 7:8], e_d[:],
            op0=ALU.is_ge, op1=ALU.mult, accum_out=s2[:, 1:2])
        nc.sync.dma_start(ds2[:], s2[:])
        lgt = act.tile([BL, 2], F32, name="lgt")
        nc.scalar.activation(lgt[:], s2[:], AF.Ln)
        res = act.tile([BL, 1], F32, name="res")
        nc.vector.tensor_sub(res[:], lgt[:, 0:1], lgt[:, 1:2])
        res2 = act.tile([BL, 1], F32, name="res2")
        nc.vector.tensor_scalar_add(res2[:], res[:], float(M1 - M2))
        nc.sync.dma_start(outp[:], res2[:, 0])

    nc.compile()
    return nc


def _tr12(a):
    a = np.ascontiguousarray(a, np.float32)
    return (a.view(np.uint32) & np.uint32(0xFFFFF000)).view(np.float32)


def _host_prep(x, z, W1, b1, W2, b2):
    # one-hot of x over the DC=512 logit rows
    oh = np.zeros((B, DC), np.float32)
    oh[np.arange(B)[:, None], np.arange(D)[None, :] * C + x] = 1.0
    ohT = np.ascontiguousarray(oh.T)                    # (512, 256)
    cbt = oh @ b2                                       # (256,)
    cbt_tail = oh[:, DC - 4 * C:] @ b2[DC - 4 * C:]     # (256,)

    k128c = np.zeros((128, K128_COLS), np.float32)
    w2t = _tr12(W2)
    for t in range(4):
        for kk in range(2):
            k128c[:, O_W2S + (t * 2 + kk) * 128:O_W2S + (t * 2 + kk + 1) * 128] = \
                w2t[kk * 128:(kk + 1) * 128, t * 128:(t + 1) * 128]
        for bt in range(2):
            k128c[:, O_OHS + (t * 2 + bt) * 128:O_OHS + (t * 2 + bt + 1) * 128] = \
                ohT[t * 128:(t + 1) * 128, bt * 128:(bt + 1) * 128]
    # tail one-hot: logits tile t=3, rows 64:128 are dc 448..511
    for bt in range(2):
        blk = np.zeros((128, 128), np.float32)
        blk[64:128, :] = ohT[448:512, bt * 128:(bt + 1) * 128]
        k128c[:, O_OHT + bt * 128:O_OHT + (bt + 1) * 128] = blk
    for t in range(4):
        g = np.zeros((128, 32), np.float32)
        g[np.arange(128), (t * 128 + np.arange(128)) // C] = 1.0
        k128c[:, O_GSEL + t * 32:O_GSEL + (t + 1) * 32] = g
    k128c[0:32, O_COEF:O_COEF + 128] = -1.0
    k128c[28:32, O_COEFT:O_COEFT + 128] = -1.0
    k128c[:, O_B1:O_B1 + 2] = b1.reshape(2, 128).T
    k128c[:, O_B2:O_B2 + 4] = b2.reshape(4, 128).T
    for bt in range(2):
        cb = cbt[bt * 128:(bt + 1) * 128]
        cbt4 = cbt_tail[bt * 128:(bt + 1) * 128]
        k128c[:, O_QOFF + bt] = (cb - NUM_LO) * QS + 0.5
        k128c[:, O_KOFF + bt] = (cbt4 + KEY_OFF) * KEY_SCALE + 0.5
    k128c[:, O_C4096] = np.int32(4096).view(np.float32)
    k128c[:, O_CFFF] = np.int32(0xFFF).view(np.float32)
    k128c[:, O_ENB] = NUM_LO - M1

    w1t = _tr12(W1)
    in_maps = []
    for c in range(P):
        kzc = np.zeros((Lz, KZ_COLS), np.float32)
        kzc[:, O_ZT:O_ZT + NL] = _tr12(z[c * NL:(c + 1) * NL, :].T)
        kzc[:, O_W1:O_W1 + H] = w1t
        in_maps.append(dict(kz=kzc, k128=k128c))
    return in_maps


_NC_CACHE = {}


def kernel(x, log_w, z, k, W1, b1, W2, b2, _trace=False, _trace_kwargs=None):
    assert int(k) == K
    in_maps = _host_prep(np.asarray(x, np.int32), np.asarray(z, np.float32),
                         np.asarray(W1, np.float32), np.asarray(b1, np.float32),
                         np.asarray(W2, np.float32), np.asarray(b2, np.float32))
    if "nc" not in _NC_CACHE:
        _NC_CACHE["nc"] = _build_nc()
    nc = _NC_CACHE["nc"]
    res = run_bass_kernel_spmd(
        nc, in_maps, list(range(P)), trace=_trace, **(_trace_kwargs or {}))
    if _trace:
        _NC_CACHE["last_result"] = res
    return np.concatenate([np.asarray(res.results[c]["out"], np.float32)
                           for c in range(P)])


# revision 10
# speedup vs baseline: 1.2999x; 1.1499x over previous
"""Trainium2 Bass kernel v3 for nn_CategoricalDecoder (topk_masking).

Bin-sharded single-pass design: each core computes full logits for its
1024 bins (f32r 1-term), derives num = full-feature logp sum and
score = tail-feature logp sum for all 256 batch rows via one-hot
matmuls, packs (20-bit fixed-point score key | 12-bit quantized num)
into positive fp32 bit patterns, and max8 extracts the per-row local
top-8 candidates WITH their num payloads in one instruction. An 8KB
AllToAll flips to batch sharding; the receiving core thresholds at the
16th-largest key and computes both logsumexps from the decoded
payloads. No z gather, no second net pass.
"""

import numpy as np
from contextlib import ExitStack

import bass_rust as _br
import concourse.bass as bass
import concourse.bacc as bacc
import concourse.tile as tile
from concourse import mybir
from concourse.bass_utils import run_bass_kernel_spmd
from concourse.hw_specs import get_activation_tables

F32 = mybir.dt.float32
F32R = mybir.dt.float32r
I32 = mybir.dt.int32
U8 = mybir.dt.uint8
AF = mybir.ActivationFunctionType
ALU = mybir.AluOpType
AX = mybir.AxisListType

B, N, Lz, H, D, C = 256, 8192, 64, 256, 32, 16
DC = D * C
P = 8
NL = N // P
BL = B // P
K = 16
NEG = -1.0

# packing constants
KEY_OFF, KEY_SCALE = 24.0, 16384.0
NUM_LO, NUM_W = -140.0, 80.0
QS = 4095.0 / NUM_W
M1, M2 = -86.0, -72.0  # fixed logsumexp shifts (num / den)

# kz column offsets (64-partition tile)
O_ZT, O_W1 = 0, NL
KZ_COLS = NL + H

# k128 column offsets (128-partition tile)
O_W2S = 0                    # 8 x [128,128] f32r: (t,kk) -> (t*2+kk)*128
O_OHS = O_W2S + 1024         # 8 x [128,128] f32r: (t,bt) -> (t*2+bt)*128
O_OHT = O_OHS + 1024         # 2 x [128,128] f32r: tail one-hot (rows 64:128)
O_GSEL = O_OHT + 256         # 4 x [128,32] f32r
O_COEF = O_GSEL + 128        # [32,128] of -1
O_COEFT = O_COEF + 128       # [32,128], -1 on rows 28:32 only
O_B1 = O_COEFT + 128         # [128,2] b1 per m
O_B2 = O_B1 + 2              # [128,4] b2 per t
O_QOFF = O_B2 + 4            # [128,2] q-affine bias per bt
O_KOFF = O_QOFF + 2          # [128,2] key-affine bias per bt
O_C4096 = O_KOFF + 2         # [128,1] int32 4096 (bit pattern)
O_CFFF = O_C4096 + 1         # [128,1] int32 0xFFF
O_ENB = O_CFFF + 1           # [128,1] f32: NUM_LO - M1 (e_n exp bias)
K128_COLS = O_ENB + 1


class _Bacc(bacc.Bacc):
    """Bacc pinning activations to the one table holding
    {Relu, Exp, Ln, Copy}, avoiding per-switch ACT_TABLE_LOADs."""

    def insert_act_table_loads(self):
        has_act = any(isinstance(i, mybir.InstActivation)
                      for b in self.main_func.blocks for i in b.instructions)
        if not has_act:
            return
        tables = []
        for name, funcs in get_activation_tables(self.m.arch).items():
            keep = funcs if name == "natural_log_exp_and_others" else set()
            tables.append((name, keep))
        _br.insert_act_table_loads(self, tables)


def _build_nc():
    nc = _Bacc("TRN2", target_bir_lowering=False, num_devices=P)

    dp = nc.declare_dram_parameter
    kz = dp("kz", [Lz, KZ_COLS], F32R, isOutput=False)
    k128 = dp("k128", [128, K128_COLS], F32R, isOutput=False)
    outp = dp("out", [BL], F32, isOutput=True)
    den_o = dp("den_o", [BL, P * 8], F32, isOutput=True)
    ded_o = dp("ded_o", [BL, P * 8], F32, isOutput=True)

    with tile.TileContext(nc) as tc, ExitStack() as ctx:
        const = ctx.enter_context(tc.tile_pool(name="const", bufs=1))
        dram = ctx.enter_context(tc.tile_pool(name="dram", bufs=1, space="DRAM"))

        kz_sb = const.tile([Lz, KZ_COLS], F32R, name="kz_sb")
        nc.sync.dma_start(kz_sb[:], kz[:])
        k128_sb = const.tile([128, K128_COLS], F32R, name="k128_sb")
        nc.sync.dma_start(k128_sb[:], k128[:])

        def c128(off, w, p=128, dt=None):
            ap = k128_sb[0:p, off:off + w]
            return ap.bitcast(dt) if dt else ap

        xin = dram.tile([B, 8], F32)
        xout = dram.tile([B, 8], F32)
        bar_in = dram.tile([1, 128], U8)
        bar_out = dram.tile([P, 128], U8)

        # early dummy collective: absorbs CC bootstrap + launch skew while
        # the parameter DMAs and phase-A matmuls run.
        nc.gpsimd.collective_compute(
            "AllGather", ALU.bypass, replica_groups=[list(range(P))],
            ins=[bar_in[:].opt()], outs=[bar_out[:].opt()],
        )

        act = ctx.enter_context(tc.tile_pool(name="act", bufs=1))
        scr = ctx.enter_context(tc.tile_pool(name="scr", bufs=6))

        # ---- h = relu(W1.T @ zT + b1): [256, NL] as 2 m-tiles ----
        hrs = []
        with ExitStack() as ctxh:
            hp = ctxh.enter_context(tc.tile_pool(name="hp", bufs=2, space="PSUM"))
            for m in range(2):
                ph = hp.tile([128, NL], F32, tag="h")
                for f in range(2):
                    sl = slice(f * 512, (f + 1) * 512)
                    nc.tensor.matmul(ph[:, sl],
                                     kz_sb[:, O_W1 + m * 128:O_W1 + (m + 1) * 128],
                                     kz_sb[:, sl], start=True, stop=True)
                hr = act.tile([128, NL], F32R, name=f"hr{m}")
                nc.scalar.activation(hr[:], ph[:], AF.Relu,
                                     bias=c128(O_B1 + m, 1, dt=F32))
                hrs.append(hr)

        # ---- per f-chunk: logits, lse, num, score, pack ----
        packed = [act.tile([128, NL], I32, name=f"pk{bt}") for bt in range(2)]
        c4096_t = act.tile([128, 512], I32, name="c4096_t")
        nc.vector.memset(c4096_t[:], 4096)
        with ExitStack() as ctxA:
            lgp = ctxA.enter_context(tc.tile_pool(name="lgp", bufs=2, space="PSUM"))
            nump = ctxA.enter_context(tc.tile_pool(name="nump", bufs=4, space="PSUM"))
            psep = ctxA.enter_context(tc.tile_pool(name="psep", bufs=2, space="PSUM"))

            for f in range(2):
                sl = slice(f * 512, (f + 1) * 512)
                pnum = [nump.tile([128, 512], F32, tag="nm", name=f"pn{f}{i}")
                        for i in range(2)]
                pscore = [nump.tile([128, 512], F32, tag="nm", name=f"ps{f}{i}")
                          for i in range(2)]
                pse = psep.tile([32, 512], F32, tag="se")
                for ti, t in enumerate([3, 0, 1, 2]):
                    lg = lgp.tile([128, 512], F32, tag="lg")
                    for kk in range(2):
                        nc.tensor.matmul(
                            lg[:], c128(O_W2S + (t * 2 + kk) * 128, 128),
                            hrs[kk][:, sl], start=(kk == 0), stop=(kk == 1))
                    e_t = scr.tile([128, 512], F32R, tag="e")
                    nc.scalar.activation(e_t[:], lg[:], AF.Exp,
                                         bias=c128(O_B2 + t, 1, dt=F32))
                    l_t = scr.tile([128, 512], F32R, tag="l")
                    nc.scalar.copy(l_t[:], lg[:])
                    nc.tensor.matmul(pse[:], c128(O_GSEL + t * 32, 32), e_t[:],
                                     start=(ti == 0), stop=(ti == 3))
                    for bt in range(2):
                        nc.tensor.matmul(pnum[bt][:],
                                         c128(O_OHS + (t * 2 + bt) * 128, 128),
                                         l_t[:], start=(ti == 0), stop=False)
                    if t == 3:
                        for bt in range(2):
                            nc.tensor.matmul(pscore[bt][:],
                                             c128(O_OHT + bt * 128, 128),
                                             l_t[:], start=True, stop=False)
                lnp = scr.tile([32, 512], F32R, tag="ln")
                nc.scalar.activation(lnp[:], pse[:], AF.Ln)
                for bt in range(2):
                    nc.tensor.matmul(pnum[bt][:], c128(O_COEF, 128, p=32),
                                     lnp[:], start=False, stop=True)
                    nc.tensor.matmul(pscore[bt][:], c128(O_COEFT, 128, p=32),
                                     lnp[:], start=False, stop=True)
                # pack: q = int(pnum*QS + qoff), key = int(pscore*KS + koff),
                # packed = key*4096 + q  (int32 domain)
                for bt in range(2):
                    q_i = scr.tile([128, 512], I32, tag="qi")
                    nc.vector.tensor_scalar(q_i[:], pnum[bt][:], QS,
                                            c128(O_QOFF + bt, 1, dt=F32),
                                            op0=ALU.mult, op1=ALU.add)
                    k_i = scr.tile([128, 512], I32, tag="ki")
                    nc.vector.tensor_scalar(k_i[:], pscore[bt][:], KEY_SCALE,
                                            c128(O_KOFF + bt, 1, dt=F32),
                                            op0=ALU.mult, op1=ALU.add)
                    k4 = scr.tile([128, 512], I32, tag="k4")
                    nc.vector.tensor_tensor(k4[:], k_i[:], c4096_t[:],
                                            op=ALU.mult)
                    nc.vector.tensor_tensor(packed[bt][:, sl], k4[:], q_i[:],
                                            op=ALU.bitwise_or)

        # ---- local top-8 by packed value; ship via AllToAll ----
        for bt in range(2):
            xv = act.tile([128, 8], F32, name=f"xv{bt}")
            nc.vector.max(xv[:], packed[bt][:].bitcast(F32))
            nc.sync.dma_start(xin[bt * 128:(bt + 1) * 128, :], xv[:])

        nc.gpsimd.collective_compute(
            "AllToAll", ALU.bypass, replica_groups=[list(range(P))],
            ins=[xin[:].opt()], outs=[xout[:].opt()],
        )

        # ---- merge: threshold at 16th largest, masked logsumexps ----
        y3 = act.tile([BL, P, 8], F32, name="y3")
        nc.sync.dma_start(y3[:], xout[:].rearrange("(s r) c -> r s c", s=P))
        y = y3[:].rearrange("r s c -> r (s c)")
        w1 = act.tile([BL, 8], F32, name="w1")
        nc.vector.max(w1[:], y)
        y2 = act.tile([BL, P * 8], F32, name="y2")
        nc.vector.match_replace(y2[:], w1[:], y, NEG)
        w2 = act.tile([BL, 8], F32, name="w2")
        nc.vector.max(w2[:], y2[:])

        u_i = y.bitcast(I32)
        cfff_t = act.tile([BL, P * 8], I32, name="cfff_t")
        nc.vector.memset(cfff_t[:], 0xFFF)
        q_d = act.tile([BL, P * 8], I32, name="qd")
        nc.vector.tensor_tensor(q_d[:], u_i, cfff_t[:], op=ALU.bitwise_and)
        km_i = act.tile([BL, P * 8], I32, name="km")
        nc.vector.tensor_tensor(km_i[:], u_i, q_d[:], op=ALU.subtract)
        q_f = act.tile([BL, P * 8], F32, name="qf")
        nc.vector.tensor_copy(q_f[:], q_d[:])
        km_f = act.tile([BL, P * 8], F32, name="kmf")
        nc.vector.tensor_copy(km_f[:], km_i[:])

        # e_n = exp(num - M1), num = q/QS + NUM_LO
        e_n = act.tile([BL, P * 8], F32, name="en")
        nc.scalar.activation(e_n[:], q_f[:], AF.Exp,
                             scale=1.0 / QS, bias=c128(O_ENB, 1, p=BL, dt=F32))
        # den - M2 = q/QS - km/(4096*KS) + (NUM_LO + KEY_OFF - M2)
        nd = act.tile([BL, P * 8], F32, name="nd")
        nc.vector.tensor_scalar(nd[:], q_f[:], 1.0 / QS,
                                float(NUM_LO + KEY_OFF - M2),
                                op0=ALU.mult, op1=ALU.add)
        dd = act.tile([BL, P * 8], F32, name="dd")
        nc.vector.scalar_tensor_tensor(
            dd[:], km_f[:], -1.0 / (4096.0 * KEY_SCALE), nd[:],
            op0=ALU.mult, op1=ALU.add)
        e_d = act.tile([BL, P * 8], F32, name="ed")
        nc.scalar.activation(e_d[:], dd[:], AF.Exp)
        nc.sync.dma_start(den_o[:], e_n[:])
        nc.sync.dma_start(ded_o[:], e_d[:])

        s2 = act.tile([BL, 2], F32, name="s2")
        junk = act.tile([BL, P * 8], F32, name="junk")
        nc.vector.scalar_tensor_tensor(
            junk[:], y, w2[:, 7:8], e_n[:],
            op0=ALU.is_ge, op1=ALU.mult, accum_out=s2[:, 0:1])
        junk2 = act.tile([BL, P * 8], F32, name="junk2")
        nc.vector.scalar_tensor_tensor(
            junk2[:], y, w2[:, 7:8], e_d[:],
            op0=ALU.is_ge, op1=ALU.mult, accum_out=s2[:, 1:2])
        lgt = act.tile([BL, 2], F32, name="lgt")
        nc.scalar.activation(lgt[:], s2[:], AF.Ln)
        res = act.tile([BL, 1], F32, name="res")
        nc.vector.tensor_sub(res[:], lgt[:, 0:1], lgt[:, 1:2])
        res2 = act.tile([BL, 1], F32, name="res2")
        nc.vector.tensor_scalar_add(res2[:], res[:], float(M1 - M2))
        nc.sync.dma_start(outp[:], res2[:, 0])

    nc.compile()
    return nc


def _tr12(a):
    a = np.ascontiguousarray(a, np.float32)
    return (a.view(np.uint32) & np.uint32(0xFFFFF000)).view(np.float32)


def _host_prep(x, z, W1, b1, W2, b2):
    # one-hot of x over the DC=512 logit rows
    oh = np.zeros((B, DC), np.float32)
    oh[np.arange(B)[:, None], np.arange(D)[None, :] * C + x] = 1.0
    ohT = np.ascontiguousarray(oh.T)                    # (512, 256)
    cbt = oh @ b2                                       # (256,)
    cbt_tail = oh[:, DC - 4 * C:] @ b2[DC - 4 * C:]     # (256,)

    k128c = np.zeros((128, K128_COLS), np.float32)
    w2t = _tr12(W2)
    for t in range(4):
        for kk in range(2):
            k128c[:, O_W2S + (t * 2 + kk) * 128:O_W2S + (t * 2 + kk + 1) * 128] = \
                w2t[kk * 128:(kk + 1) * 128, t * 128:(t + 1) * 128]
        for bt in range(2):
            k128c[:, O_OHS + (t * 2 + bt) * 128:O_OHS + (t * 2 + bt + 1) * 128] = \
                ohT[t * 128:(t + 1) * 128, bt * 128:(bt + 1) * 128]
    # tail one-hot: logits tile t=3, rows 64:128 are dc 448..511
    for bt in range(2):
        blk = np.zeros((128, 128), np.float32)
        blk[64:128, :] = ohT[448:512, bt * 128:(bt + 1) * 128]
        k128c[:, O_OHT + bt * 128:O_OHT + (bt + 1) * 128] = blk
    for t in range(4):
        g = np.zeros((128, 32), np.float32)
        g[np.arange(128), (t * 128 + np.arange(128)) // C] = 1.0
        k128c[:, O_GSEL + t * 32:O_GSEL + (t + 1) * 32] = g
    k128c[0:32, O_COEF:O_COEF + 128] = -1.0
    k128c[28:32, O_COEFT:O_COEFT + 128] = -1.0
    k128c[:, O_B1:O_B1 + 2] = b1.reshape(2, 128).T
    k128c[:, O_B2:O_B2 + 4] = b2.reshape(4, 128).T
    for bt in range(2):
        cb = cbt[bt * 128:(bt + 1) * 128]
        cbt4 = cbt_tail[bt * 128:(bt + 1) * 128]
        k128c[:, O_QOFF + bt] = (cb - NUM_LO) * QS + 0.5
        k128c[:, O_KOFF + bt] = (cbt4 + KEY_OFF) * KEY_SCALE + 0.5
    k128c[:, O_C4096] = np.int32(4096).view(np.float32)
    k128c[:, O_CFFF] = np.int32(0xFFF).view(np.float32)
    k128c[:, O_ENB] = NUM_LO - M1

    w1t = _tr12(W1)
    in_maps = []
    for c in range(P):
        kzc = np.zeros((Lz, KZ_COLS), np.float32)
        kzc[:, O_ZT:O_ZT + NL] = _tr12(z[c * NL:(c + 1) * NL, :].T)
        kzc[:, O_W1:O_W1 + H] = w1t
        in_maps.append(dict(kz=kzc, k128=k128c))
    return in_maps


_NC_CACHE = {}


def kernel(x, log_w, z, k, W1, b1, W2, b2, _trace=False, _trace_kwargs=None):
    assert int(k) == K
    in_maps = _host_prep(np.asarray(x, np.int32), np.asarray(z, np.float32),
                         np.asarray(W1, np.float32), np.asarray(b1, np.float32),
                         np.asarray(W2, np.float32), np.asarray(b2, np.float32))
    if "nc" not in _NC_CACHE:
        _NC_CACHE["nc"] = _build_nc()
    nc = _NC_CACHE["nc"]
    res = run_bass_kernel_spmd(
        nc, in_maps, list(range(P)), trace=_trace, **(_trace_kwargs or {}))
    if _trace:
        _NC_CACHE["last_result"] = res
    return np.concatenate([np.asarray(res.results[c]["out"], np.float32)
                           for c in range(P)])
